# revision 23
# baseline (speedup 1.0000x reference)
"""GCN message-passing layer (4x GCNConv sum) on 8 Trainium2 NeuronCores.

out[d] = sum_i ( segment_sum_{e in E_i, dst=d} x[src_e] ) @ W_i

Self-contained kernel: takes FULL inputs, shards destination nodes across the
8 cores (graph parallel, x replicated), runs one SPMD Bass/Tile program via
run_bass_kernel_spmd, returns the FULL [N, H] output.

v4: HBM dma_gather, drain-floor tuned. The SDMA engines sustain ~30ns per
256B gather packet, so wall time ~ gathered_rows * 30ns / 16 engines; the
design minimizes REAL gathered rows and keeps all 4 SWDGE queues fed:

  - Groups keyed (bank, set, dst-tile(128)), sized to the 16-aligned
    cross-core max. Pad index slots hold -1: the Q7 desc-gen trims trailing
    negatives per core at runtime, so pad rows cost no descriptors and no
    drain; their staging slots stay stale and the one-hot S rows are -1
    (select no column), keeping the matmul exact.
  - One dma_gather per group (~550 rows, single_packet=True), queues
    round-robin so the 4 Q7 core pairs generate descriptors concurrently;
    dynamic_dma_scratch_size=49152 gives 768-descriptor rings.
  - idx/dstf DMAs, one-hot S builds (VectorE is_equal vs iota, bf16) and
    staging tiles are chunked (~32 staging cols) with deep pools (bufs=6)
    so gathers run ahead of TensorE consumption (the v1/v2 serializer).
  - Per (set, bank, tile): matmul pieces accumulate g^T[h,128d] in PSUM;
    ACT copies PSUM->SBUF bf16; one matmul vs W_i (bf16) -> opsum; VectorE
    adds opsum into the fp32 out_sb accumulator. No cross-phase PSUM
    residency, no phase-2 barrier.
  - Final: out_sb -> DRAM per tile.
"""
import math
import sys

sys.path.insert(0, "/opt/trn_rl_repo")

import numpy as np
import ml_dtypes

from concourse import bass, mybir, tile, bacc
from concourse.bass_utils import run_bass_kernel_spmd

P = 128
N_CORES = 8
DT = 128          # dst-tile width (= one-hot S width = psum tile width)
BANK_ROWS = 32768
N_SETS = 4
CHUNK_COLS = 32   # staging columns per chunk (S build / stg tile unit)


class _Cfg:
    def __init__(self, n_nodes):
        self.n_nodes = n_nodes
        self.npc = n_nodes // N_CORES
        self.nt = math.ceil(self.npc / DT)
        self.n_banks = math.ceil(n_nodes / BANK_ROWS)
        self.x_rows_pad = self.n_banks * BANK_ROWS


def _host_prep(cfg, edges_list):
    NC, NS, NT, NB = N_CORES, N_SETS, cfg.nt, cfg.n_banks
    cnt = np.zeros((NC, NS, NT, NB), np.int64)
    per_set = []
    for i, e in enumerate(edges_list):
        src = np.asarray(e[0], np.int64)
        dst = np.asarray(e[1], np.int64)
        core = dst // cfg.npc
        dloc = dst % cfg.npc
        t = dloc // DT
        b = src // BANK_ROWS
        key = (core * NT + t) * NB + b
        cnt[:, i] = np.bincount(key, minlength=NC * NT * NB).reshape(NC, NT, NB)
        order = np.argsort(key, kind="stable")
        per_set.append((src[order], dloc[order], key[order]))

    # 16-aligned cross-core-max rows per (set, tile, bank); one dma_gather
    # per group (num_idxs exact), staging gets ceil(G/128) columns whose tail
    # is stale - stale rows carry dstf=-1 (select no column).
    G = ((cnt.max(axis=0) + 15) // 16) * 16            # [NS, NT, NB]
    Gcols = -(-G // P)                                  # staging cols per group

    # groups ordered (bank, set, tile)
    i16_off = np.zeros((NS, NT, NB), np.int64)   # idx offset, 16-slot units
    col_off = np.zeros((NS, NT, NB), np.int64)   # staging-col offset
    i16 = 0
    col = 0
    for b in range(NB):
        for i in range(NS):
            for t in range(NT):
                i16_off[i, t, b] = i16
                col_off[i, t, b] = col
                i16 += G[i, t, b] // 16
                col += Gcols[i, t, b]
    tot_i16 = i16
    tot_cols = col

    idx_mats, dstf_mats = [], []
    for c in range(NC):
        idx_flat = np.zeros(tot_i16 * 16, np.int64)   # pad slots gather row 0
        dstf_flat = np.full(tot_cols * P, -1.0, np.float32)
        for i in range(NS):
            src_s, dloc_s, key_s = per_set[i]
            lo = np.searchsorted(key_s, c * NT * NB)
            hi = np.searchsorted(key_s, (c + 1) * NT * NB)
            src_c, dloc_c, key_c = src_s[lo:hi], dloc_s[lo:hi], key_s[lo:hi]
            t_c = (key_c // NB) % NT
            b_c = key_c % NB
            gstart = np.searchsorted(key_c, key_c)
            rank = np.arange(len(key_c)) - gstart
            idx_flat[i16_off[i, t_c, b_c] * 16 + rank] = src_c - b_c * BANK_ROWS
            dstf_flat[col_off[i, t_c, b_c] * P + rank] = (
                dloc_c - t_c * DT).astype(np.float32)
        idx16 = idx_flat.reshape(tot_i16, 16).T.astype(np.int16)   # [16, tot_i16]
        idx_mats.append(np.tile(idx16, (8, 1)))                    # [128, tot_i16]
        dstf_mats.append(dstf_flat.reshape(tot_cols, P).T.astype(ml_dtypes.bfloat16))

    # chunks: per (b, i), consecutive tiles with staging cols <= CHUNK_COLS
    chunks = {}
    for b in range(NB):
        for i in range(NS):
            lst = []
            t = 0
            while t < NT:
                t0 = t
                cols_acc = 0
                while t < NT and cols_acc + int(Gcols[i, t, b]) <= CHUNK_COLS:
                    cols_acc += int(Gcols[i, t, b])
                    t += 1
                if t == t0:
                    cols_acc = int(Gcols[i, t, b])
                    t += 1
                lst.append((t0, t, cols_acc))
            chunks[(b, i)] = lst

    return dict(G=G, Gcols=Gcols, i16_off=i16_off, col_off=col_off,
                tot_i16=tot_i16, tot_cols=tot_cols, idx_mats=idx_mats,
                dstf_mats=dstf_mats, chunks=chunks)


def _build_kernel(cfg, prep):
    NS, NT, NB = N_SETS, cfg.nt, cfg.n_banks
    G, Gcols = prep["G"], prep["Gcols"]
    i16_off, col_off = prep["i16_off"], prep["col_off"]
    tot_i16, tot_cols = prep["tot_i16"], prep["tot_cols"]
    chunks = prep["chunks"]
    msg_dt = mybir.dt.bfloat16

    max_ck_cols = max(c for lst in chunks.values() for (_, _, c) in lst)
    max_ck_i16 = max_ck_cols * 8

    nc = bacc.Bacc("TRN2", target_bir_lowering=False, debug=False,
                   num_devices=N_CORES, num_swdge_queues=4,
                   dynamic_dma_scratch_size=49152)
    x = nc.dram_tensor("x", [cfg.x_rows_pad, P], msg_dt,
                       kind="ExternalInput").ap()
    idx_d = nc.dram_tensor("idx", [P, tot_i16], mybir.dt.int16,
                           kind="ExternalInput").ap()
    dstf_d = nc.dram_tensor("dstf", [P, tot_cols], mybir.dt.bfloat16,
                            kind="ExternalInput").ap()
    iota_d = nc.dram_tensor("iota", [P, DT], msg_dt, kind="ExternalInput").ap()
    w_d = nc.dram_tensor("w", [NS * P, P], mybir.dt.bfloat16,
                         kind="ExternalInput").ap()
    out_d = nc.dram_tensor("out", [cfg.npc, P], mybir.dt.float32,
                           kind="ExternalOutput").ap()

    with tile.TileContext(nc) as tc:
        with tc.tile_pool(name="const", bufs=1) as constp, \
             tc.tile_pool(name="acc", bufs=1) as accp, \
             tc.tile_pool(name="idxp", bufs=10) as idxp, \
             tc.tile_pool(name="dstfp", bufs=10) as dstfp, \
             tc.tile_pool(name="stg", bufs=7) as stgp, \
             tc.tile_pool(name="sp", bufs=7) as spool, \
             tc.tile_pool(name="gt", bufs=8) as gtp, \
             tc.tile_pool(name="gps", bufs=6, space="PSUM") as gpsp, \
             tc.tile_pool(name="ops", bufs=2, space="PSUM") as opsp:

            iota_sb = constp.tile([P, 1, DT], msg_dt)
            nc.sync.dma_start(out=iota_sb[:, 0, :], in_=iota_d[:])
            w_sb = constp.tile([P, NS, P], mybir.dt.bfloat16)
            for i in range(NS):
                nc.sync.dma_start(out=w_sb[:, i, :], in_=w_d[i * P:(i + 1) * P, :])

            out_sb = accp.tile([P, NT, P], mybir.dt.float32)
            nc.vector.memset(out_sb[:], 0.0)

            gq = [0]
            # prime the staging pool: stale (ungathered) column tails feed
            # matmuls with S=0; initial SBUF garbage can be NaN and NaN*0=NaN,
            # so zero the rotating buffers once. Afterwards stale bytes are
            # always previously gathered (finite) x rows.
            for _ in range(7):
                z = stgp.tile([P, max_ck_cols, P], msg_dt, tag="stg", name="stg")
                nc.vector.memset(z[:], 0.0)

            for b in range(NB):
                for i in range(NS):
                    for (t0, t1, ck_cols) in chunks[(b, i)]:
                        c0 = int(col_off[i, t0, b])
                        i0 = int(i16_off[i, t0, b])
                        n16 = (int(i16_off[i, t1 - 1, b])
                               + int(G[i, t1 - 1, b]) // 16 - i0)
                        idx_sb = idxp.tile([P, max_ck_i16], mybir.dt.int16,
                                           tag="idx", name="idx")
                        nc.sync.dma_start(out=idx_sb[:, :n16],
                                          in_=idx_d[:, i0:i0 + n16])
                        dstf_sb = dstfp.tile([P, max_ck_cols, 1],
                                             mybir.dt.bfloat16, tag="dstf",
                                             name="dstf")
                        nc.sync.dma_start(out=dstf_sb[:, :ck_cols, 0],
                                          in_=dstf_d[:, c0:c0 + ck_cols])
                        s_group = spool.tile([P, max_ck_cols, DT], msg_dt,
                                             tag="sg", name="sg")
                        nc.vector.tensor_tensor(
                            out=s_group[:, :ck_cols, :],
                            in0=dstf_sb[:, :ck_cols, :].to_broadcast(
                                [P, ck_cols, DT]),
                            in1=iota_sb[:].to_broadcast([P, ck_cols, DT]),
                            op=mybir.AluOpType.is_equal)

                        stg = stgp.tile([P, max_ck_cols, P], msg_dt,
                                        tag="stg", name="stg")
                        for t in range(t0, t1):
                            g = int(G[i, t, b])
                            if g == 0:
                                continue
                            lc0 = int(col_off[i, t, b]) - c0
                            li0 = int(i16_off[i, t, b]) - i0
                            nc.gpsimd.dma_gather(
                                out_ap=stg[:, lc0:lc0 + int(Gcols[i, t, b]), :],
                                in_ap=x[b * BANK_ROWS:(b + 1) * BANK_ROWS, :],
                                idxs_ap=idx_sb[:, li0:li0 + g // 16],
                                num_idxs=g,
                                num_idxs_reg=g,
                                elem_size=P,
                                single_packet=True,
                                queue_num=gq[0] % 4,
                            )
                            gq[0] += 1

                        for t in range(t0, t1):
                            g = int(G[i, t, b])
                            if g == 0:
                                continue
                            lc0 = int(col_off[i, t, b]) - c0
                            ncol = int(Gcols[i, t, b])
                            gpsum = gpsp.tile([P, P], mybir.dt.float32,
                                              space="PSUM", tag="gp", name="gp")
                            for k in range(ncol):
                                nc.tensor.matmul(
                                    out=gpsum[:],
                                    lhsT=stg[:, lc0 + k, :],
                                    rhs=s_group[:, lc0 + k, :],
                                    start=(k == 0), stop=(k == ncol - 1))
                            gtmp = gtp.tile([P, P], msg_dt, tag="gt", name="gt")
                            nc.scalar.copy(out=gtmp[:], in_=gpsum[:])
                            opsum = opsp.tile([P, P], mybir.dt.float32,
                                              space="PSUM", tag="o", name="o")
                            nc.tensor.matmul(
                                out=opsum[:],
                                lhsT=gtmp[:],
                                rhs=w_sb[:, i, :],
                                start=True, stop=True)
                            nc.vector.tensor_tensor(
                                out=out_sb[:, t, :],
                                in0=opsum[:],
                                in1=out_sb[:, t, :],
                                op=mybir.AluOpType.add)

            for t in range(NT):
                d0 = t * DT
                rows = min(DT, cfg.npc - d0)
                if rows <= 0:
                    continue
                nc.sync.dma_start(out=out_d[d0:d0 + rows, :],
                                  in_=out_sb[:rows, t, :])
    nc.compile()
    return nc


def _prepare(hidden_states, edges_i, edges_ii, edges_iii, edges_a,
             W_i, W_ii, W_iii, W_a):
    x = np.asarray(hidden_states, np.float32)
    n_nodes = x.shape[0]
    cfg = _Cfg(n_nodes)
    edges_list = [np.asarray(e) for e in (edges_i, edges_ii, edges_iii, edges_a)]
    w_list = [np.asarray(w, np.float32) for w in (W_i, W_ii, W_iii, W_a)]

    prep = _host_prep(cfg, edges_list)
    nc = _build_kernel(cfg, prep)

    x_pad = np.zeros((cfg.x_rows_pad, P), np.float32)
    x_pad[:n_nodes] = x
    x_pad = x_pad.astype(ml_dtypes.bfloat16)
    iota = np.tile(np.arange(DT, dtype=np.float32)[None, :], (P, 1)).astype(
        ml_dtypes.bfloat16)
    w_cat = np.concatenate(w_list, axis=0).astype(ml_dtypes.bfloat16)

    in_maps = [{
        "x": x_pad,
        "idx": prep["idx_mats"][c],
        "dstf": prep["dstf_mats"][c],
        "iota": iota,
        "w": w_cat,
    } for c in range(N_CORES)]
    return nc, in_maps


def kernel(hidden_states, edges_i, edges_ii, edges_iii, edges_a,
           W_i, W_ii, W_iii, W_a):
    nc, in_maps = _prepare(hidden_states, edges_i, edges_ii, edges_iii,
                           edges_a, W_i, W_ii, W_iii, W_a)
    res = run_bass_kernel_spmd(nc, in_maps, core_ids=list(range(N_CORES)))
    out = np.concatenate([res.results[c]["out"] for c in range(N_CORES)], axis=0)
    return out.astype(np.float32)
